# revision 6
# baseline (speedup 1.0000x reference)
"""CLUB loss kernel for Trainium2 (8 NeuronCores, SPMD).

Math
----
Reference computes, with flat_x = transpose(x,(0,2,3,1)).reshape(N,d),
ie = exp(-p_logvar):

  positive[i] = -0.5 * sum_d (x_i - mu_i)^2 * ie_i
  negative[i] = -0.5 * mean_j [ <x_j^2, ie_i> - 2 <x_j, mu_i*ie_i> + <mu_i^2, ie_i> ]
  loss = mean_i (positive - negative)

Because the loss only needs mean_j of a form affine in (x_j, x_j^2), the
(N,N) distance matrix collapses:

  sum_j D[i,j] = <ie_i, Sx2> - 2 <mu_i*ie_i, Sx> + N*<mu_i^2, ie_i>
  with Sx = sum_j x_j, Sx2 = sum_j x_j^2   (d-vectors, global over all rows)

So each core k (rows [784k, 784k+784) == batch element k) reduces its slab to
seven per-channel vectors (free-axis reductions in a channels-on-partitions
layout):

  A    = sum_i ie_i            B    = sum_i mu_i*ie_i
  Sx   = sum_i x_i             Sx2  = sum_i x_i^2
  Px2  = sum_i x_i^2*ie_i      Pxbm = sum_i x_i*mu_i*ie_i
  Cv   = sum_i mu_i^2*ie_i

and the host combines:
  P    = sum(Px2) - 2*sum(Pxbm) + sum(Cv)              # sum_i -2*positive[i]
  neg  = <A, Sx2_g> - 2 <B, Sx_g> + N*sum(Cv_g)        # sum_{i,j} D[i,j]
  loss = (-0.5*P + 0.5/N * neg) / N

Sharding: data-parallel over rows i; x arrives in DRAM already channels-major
per batch element (x[k] is (512, 784) == flat_x-slab transposed), so the
device kernel needs no transposes. mu/logvar slabs are transposed on the host
during input marshalling.
"""

import numpy as np

B, D, H, W = 8, 512, 28, 28
N = B * H * W            # 6272 rows
NCORES = 8
ROWS = N // NCORES       # 784 rows per core == H*W
NT = D // 128            # 4 channel tiles of 128 partitions
NSTAT = 7                # A, B, Sx, Sx2, Px2, Pxbm, Cv

_PROGRAM_CACHE: dict = {}


def build_program():
    """One Bass program, broadcast SPMD to all 8 cores (data differs per core).

    Raw Bass (no TileContext): this walrus build rejects Tile's drain tail
    ("Too many sync wait commands") and the InstTensorTensorReduce encoding
    ("ISA wrong length"), so sync is managed manually and the fused
    multiply+reduce uses scalar_tensor_tensor (which encodes fine).
    """
    from contextlib import ExitStack

    import concourse.bass as bass
    import concourse.mybir as mybir

    f32 = mybir.dt.float32
    Alu = mybir.AluOpType
    Act = mybir.ActivationFunctionType

    nc = bass.Bass()
    xT = nc.declare_dram_parameter("xT", [D, ROWS], f32, isOutput=False)
    muT = nc.declare_dram_parameter("muT", [D, ROWS], f32, isOutput=False)
    lvT = nc.declare_dram_parameter("lvT", [D, ROWS], f32, isOutput=False)
    stats = nc.declare_dram_parameter("stats", [128, NSTAT * NT], f32, isOutput=True)

    with ExitStack() as ctx:
        sb = lambda name, shape: ctx.enter_context(nc.sbuf_tensor(name, shape, f32))
        x = [sb(f"x{t}", [128, ROWS]) for t in range(NT)]
        mu = [sb(f"mu{t}", [128, ROWS]) for t in range(NT)]
        lv = [sb(f"lv{t}", [128, ROWS]) for t in range(NT)]
        ie = [sb(f"ie{t}", [128, ROWS]) for t in range(NT)]
        xs = [sb(f"xs{t}", [128, ROWS]) for t in range(NT)]
        bm = [sb(f"bm{t}", [128, ROWS]) for t in range(NT)]
        sc_act = sb("sc_act", [128, ROWS])   # dead ACT outs (ACT is in-order)
        sc_dve = sb("sc_dve", [128, ROWS])   # dead DVE outs (DVE is in-order)
        st = sb("st", [128, NSTAT * NT])

        sem_x = [ctx.enter_context(nc.semaphore(f"sx{t}")) for t in range(NT)]
        sem_mu = [ctx.enter_context(nc.semaphore(f"sm{t}")) for t in range(NT)]
        sem_lv = [ctx.enter_context(nc.semaphore(f"sl{t}")) for t in range(NT)]
        act_sem = ctx.enter_context(nc.semaphore("act"))
        dve_sem = ctx.enter_context(nc.semaphore("dve"))
        out_sem = ctx.enter_context(nc.semaphore("outs"))
        block = ctx.enter_context(nc.Block())

        def col(s, t):
            c = s * NT + t
            return st[:, c : c + 1]

        @block.gpsimd
        def _(gpsimd):
            for t in range(NT):
                sl_ = slice(128 * t, 128 * (t + 1))
                gpsimd.dma_start(x[t][:], xT[sl_, :]).then_inc(sem_x[t], 16)
                gpsimd.dma_start(lv[t][:], lvT[sl_, :]).then_inc(sem_lv[t], 16)
                gpsimd.dma_start(mu[t][:], muT[sl_, :]).then_inc(sem_mu[t], 16)
            gpsimd.wait_ge(act_sem, 3 * NT)
            gpsimd.wait_ge(dve_sem, 4 * NT)
            gpsimd.dma_start(stats[:, :], st[:]).then_inc(out_sem, 16)
            gpsimd.wait_ge(out_sem, 16)

        @block.scalar
        def _(scalar):
            for t in range(NT):
                scalar.wait_ge(sem_lv[t], 16)
                # ie = exp(-lv); accum -> A
                nc.scalar.activation(ie[t][:], lv[t][:], Act.Exp, bias=0.0,
                                     scale=-1.0, accum_out=col(0, t)
                                     ).then_inc(act_sem, 1)
                scalar.wait_ge(sem_x[t], 16)
                # xs = x^2; accum -> Sx2
                nc.scalar.activation(xs[t][:], x[t][:], Act.Square,
                                     accum_out=col(3, t)).then_inc(act_sem, 1)
                if t > 0:
                    scalar.wait_ge(act_sem, 3 * t)   # sc_act WAW (self, trivial)
                # copy of x (dead); accum -> Sx
                nc.scalar.activation(sc_act[:], x[t][:], Act.Copy,
                                     accum_out=col(2, t)).then_inc(act_sem, 1)

        @block.vector
        def _(vector):
            for t in range(NT):
                vector.wait_ge(act_sem, 3 * t + 1)   # ie[t] ready
                vector.wait_ge(sem_mu[t], 16)
                # bm = mu*ie; accum -> B
                nc.vector.scalar_tensor_tensor(
                    bm[t][:], mu[t][:], 1.0, ie[t][:], Alu.mult, Alu.mult,
                    accum_out=col(1, t)).then_inc(dve_sem, 1)
                vector.wait_ge(act_sem, 3 * t + 2)   # xs[t] (and x[t]) ready
                if t > 0:
                    vector.wait_ge(dve_sem, 4 * t)   # sc_dve WAW (self, trivial)
                # xs*ie (dead); accum -> Px2
                nc.vector.scalar_tensor_tensor(
                    sc_dve[:], xs[t][:], 1.0, ie[t][:], Alu.mult, Alu.mult,
                    accum_out=col(4, t)).then_inc(dve_sem, 1)
                vector.wait_ge(dve_sem, 4 * t + 2)   # bm[t] RAW + sc_dve WAW (self)
                # x*bm (dead); accum -> Pxbm
                nc.vector.scalar_tensor_tensor(
                    sc_dve[:], x[t][:], 1.0, bm[t][:], Alu.mult, Alu.mult,
                    accum_out=col(5, t)).then_inc(dve_sem, 1)
                vector.wait_ge(dve_sem, 4 * t + 3)   # sc_dve WAW (self)
                # mu*bm (dead); accum -> Cv
                nc.vector.scalar_tensor_tensor(
                    sc_dve[:], mu[t][:], 1.0, bm[t][:], Alu.mult, Alu.mult,
                    accum_out=col(6, t)).then_inc(dve_sem, 1)

    return nc


def get_program():
    if "nc" not in _PROGRAM_CACHE:
        _PROGRAM_CACHE["nc"] = build_program()
    return _PROGRAM_CACHE["nc"]


def make_in_maps(x, p_mu, p_logvar):
    """Shard full inputs into per-core input maps (data-parallel over rows)."""
    x = np.asarray(x, dtype=np.float32)
    p_mu = np.asarray(p_mu, dtype=np.float32)
    p_logvar = np.asarray(p_logvar, dtype=np.float32)
    xk = x.reshape(NCORES, D, ROWS)  # core k's slab of flat_x, transposed
    in_maps = []
    for k in range(NCORES):
        rows = slice(ROWS * k, ROWS * (k + 1))
        in_maps.append({
            "xT": np.ascontiguousarray(xk[k]),
            "muT": np.ascontiguousarray(p_mu[rows].T),
            "lvT": np.ascontiguousarray(p_logvar[rows].T),
        })
    return in_maps


def _unpack_stats(stats_arr):
    """(128, 7*NT) device layout -> (7, 512) per-channel stat vectors."""
    out = np.empty((NSTAT, D), dtype=np.float64)
    for s in range(NSTAT):
        sub = stats_arr[:, s * NT : (s + 1) * NT]  # (128, NT); sub[p, t] = v[t*128+p]
        out[s] = sub.T.reshape(D).astype(np.float64)
    return out


def combine(stats_per_core):
    """Host epilogue: all-reduce the per-core stat vectors and form the scalar."""
    tot = np.zeros((NSTAT, D), dtype=np.float64)
    for arr in stats_per_core:
        tot += _unpack_stats(arr)
    A, Bv, Sx, Sx2, Px2, Pxbm, Cv = tot
    Csum = Cv.sum()
    P = Px2.sum() - 2.0 * Pxbm.sum() + Csum       # sum_i sum_d (x-mu)^2*ie
    neg = A @ Sx2 - 2.0 * (Bv @ Sx) + N * Csum    # sum_{i,j} D[i,j]
    loss = (-0.5 * P + 0.5 / N * neg) / N
    return np.float32(loss)


def run_on_device(in_maps, trace=False, **kwargs):
    from concourse.bass_utils import run_bass_kernel_spmd

    nc = get_program()
    return run_bass_kernel_spmd(nc, in_maps, list(range(NCORES)), trace=trace,
                                **kwargs)


def kernel(x, p_mu, p_logvar):
    in_maps = make_in_maps(x, p_mu, p_logvar)
    br = run_on_device(in_maps)
    return combine([r["stats"] for r in br.results])


# revision 13
# speedup vs baseline: 1.1038x; 1.1038x over previous
"""CLUB loss kernel for Trainium2 (8 NeuronCores, SPMD).

Math
----
Reference computes, with flat_x = transpose(x,(0,2,3,1)).reshape(N,d),
ie = exp(-p_logvar):

  positive[i] = -0.5 * sum_d (x_i - mu_i)^2 * ie_i
  negative[i] = -0.5 * mean_j [ <x_j^2, ie_i> - 2 <x_j, mu_i*ie_i> + <mu_i^2, ie_i> ]
  loss = mean_i (positive - negative)

Because the loss only needs mean_j of a form affine in (x_j, x_j^2), the
(N,N) distance matrix collapses:

  sum_j D[i,j] = <ie_i, Sx2> - 2 <mu_i*ie_i, Sx> + N*<mu_i^2, ie_i>
  with Sx = sum_j x_j, Sx2 = sum_j x_j^2   (d-vectors, global over all rows)

So each core k (rows [784k, 784k+784) == batch element k) reduces its slab to
seven per-channel vectors (free-axis reductions in a channels-on-partitions
layout):

  A    = sum_i ie_i            B    = sum_i mu_i*ie_i
  Sx   = sum_i x_i             Sx2  = sum_i x_i^2
  Px2  = sum_i x_i^2*ie_i      Pxbm = sum_i x_i*mu_i*ie_i
  Cv   = sum_i mu_i^2*ie_i

and the host combines:
  P    = sum(Px2) - 2*sum(Pxbm) + sum(Cv)              # sum_i -2*positive[i]
  neg  = <A, Sx2_g> - 2 <B, Sx_g> + N*sum(Cv_g)        # sum_{i,j} D[i,j]
  loss = (-0.5*P + 0.5/N * neg) / N

Sharding: data-parallel over rows i; x arrives in DRAM already channels-major
per batch element (x[k] is (512, 784) == flat_x-slab transposed), so the
device kernel needs no transposes. mu/logvar slabs are transposed on the host
during input marshalling.
"""

import numpy as np

B, D, H, W = 8, 512, 28, 28
N = B * H * W            # 6272 rows
NCORES = 8
ROWS = N // NCORES       # 784 rows per core == H*W
NT = D // 128            # 4 channel tiles of 128 partitions
NSTAT = 7                # A, B, Sx, Sx2, Px2, Pxbm, Cv

_PROGRAM_CACHE: dict = {}


def build_program():
    """One Bass program, broadcast SPMD to all 8 cores (data differs per core).

    Raw Bass (no TileContext): this walrus build rejects Tile's drain tail
    ("Too many sync wait commands") and the InstTensorTensorReduce encoding
    ("ISA wrong length"), so sync is managed manually and the fused
    multiply+reduce uses scalar_tensor_tensor (which encodes fine).
    """
    from contextlib import ExitStack

    import concourse.bass as bass
    import concourse.mybir as mybir

    f32 = mybir.dt.float32
    Alu = mybir.AluOpType
    Act = mybir.ActivationFunctionType

    nc = bass.Bass()
    xT = nc.declare_dram_parameter("xT", [D, ROWS], f32, isOutput=False)
    muT = nc.declare_dram_parameter("muT", [D, ROWS], f32, isOutput=False)
    lvT = nc.declare_dram_parameter("lvT", [D, ROWS], f32, isOutput=False)
    stats = nc.declare_dram_parameter("stats", [128, NSTAT * NT], f32, isOutput=True)

    with ExitStack() as ctx:
        sb = lambda name, shape: ctx.enter_context(nc.sbuf_tensor(name, shape, f32))
        x = [sb(f"x{t}", [128, ROWS]) for t in range(NT)]
        mu = [sb(f"mu{t}", [128, ROWS]) for t in range(NT)]
        lv = [sb(f"lv{t}", [128, ROWS]) for t in range(NT)]
        ie = [sb(f"ie{t}", [128, ROWS]) for t in range(NT)]
        xs = [sb(f"xs{t}", [128, ROWS]) for t in range(NT)]
        bm = [sb(f"bm{t}", [128, ROWS]) for t in range(NT)]
        sc_act = sb("sc_act", [128, ROWS])   # dead ACT outs (ACT is in-order)
        sc_dve = sb("sc_dve", [128, ROWS])   # dead DVE outs (DVE is in-order)
        st = sb("st", [128, NSTAT * NT])

        sem_x = [ctx.enter_context(nc.semaphore(f"sx{t}")) for t in range(NT)]
        sem_mu = [ctx.enter_context(nc.semaphore(f"sm{t}")) for t in range(NT)]
        sem_lv = [ctx.enter_context(nc.semaphore(f"sl{t}")) for t in range(NT)]
        act_sem = ctx.enter_context(nc.semaphore("act"))
        dve_sem = ctx.enter_context(nc.semaphore("dve"))
        out_sem = ctx.enter_context(nc.semaphore("outs"))
        block = ctx.enter_context(nc.Block())

        def col(s, t):
            c = s * NT + t
            return st[:, c : c + 1]

        @block.sync
        def _(sync):
            # HWDGE input DMAs: cheap issue on the otherwise-idle SP engine.
            # Order lv,x,mu per tile so ACT's exp can start earliest.
            for t in range(NT):
                sl_ = slice(128 * t, 128 * (t + 1))
                sync.dma_start(lv[t][:], lvT[sl_, :]).then_inc(sem_lv[t], 16)
                sync.dma_start(x[t][:], xT[sl_, :]).then_inc(sem_x[t], 16)
                sync.dma_start(mu[t][:], muT[sl_, :]).then_inc(sem_mu[t], 16)
            sync.wait_ge(act_sem, 3 * NT)
            sync.wait_ge(dve_sem, 4 * NT)
            sync.dma_start(stats[:, :], st[:]).then_inc(out_sem, 16)
            sync.wait_ge(out_sem, 16)

        @block.scalar
        def _(scalar):
            for t in range(NT):
                scalar.wait_ge(sem_lv[t], 16)
                # ie = exp(-lv); accum -> A
                nc.scalar.activation(ie[t][:], lv[t][:], Act.Exp, bias=0.0,
                                     scale=-1.0, accum_out=col(0, t)
                                     ).then_inc(act_sem, 1)
                scalar.wait_ge(sem_x[t], 16)
                # xs = x^2; accum -> Sx2
                nc.scalar.activation(xs[t][:], x[t][:], Act.Square,
                                     accum_out=col(3, t)).then_inc(act_sem, 1)
                if t > 0:
                    scalar.wait_ge(act_sem, 3 * t)   # sc_act WAW (self, trivial)
                # copy of x (dead); accum -> Sx
                nc.scalar.activation(sc_act[:], x[t][:], Act.Copy,
                                     accum_out=col(2, t)).then_inc(act_sem, 1)

        @block.vector
        def _(vector):
            for t in range(NT):
                vector.wait_ge(act_sem, 3 * t + 1)   # ie[t] ready
                vector.wait_ge(sem_mu[t], 16)
                # bm = mu*ie; accum -> B
                nc.vector.scalar_tensor_tensor(
                    bm[t][:], mu[t][:], 1.0, ie[t][:], Alu.mult, Alu.mult,
                    accum_out=col(1, t)).then_inc(dve_sem, 1)
                vector.wait_ge(act_sem, 3 * t + 2)   # xs[t] (and x[t]) ready
                if t > 0:
                    vector.wait_ge(dve_sem, 4 * t)   # sc_dve WAW (self, trivial)
                # xs*ie (dead); accum -> Px2
                nc.vector.scalar_tensor_tensor(
                    sc_dve[:], xs[t][:], 1.0, ie[t][:], Alu.mult, Alu.mult,
                    accum_out=col(4, t)).then_inc(dve_sem, 1)
                vector.wait_ge(dve_sem, 4 * t + 2)   # bm[t] RAW + sc_dve WAW (self)
                # x*bm (dead); accum -> Pxbm
                nc.vector.scalar_tensor_tensor(
                    sc_dve[:], x[t][:], 1.0, bm[t][:], Alu.mult, Alu.mult,
                    accum_out=col(5, t)).then_inc(dve_sem, 1)
                vector.wait_ge(dve_sem, 4 * t + 3)   # sc_dve WAW (self)
                # mu*bm (dead); accum -> Cv
                nc.vector.scalar_tensor_tensor(
                    sc_dve[:], mu[t][:], 1.0, bm[t][:], Alu.mult, Alu.mult,
                    accum_out=col(6, t)).then_inc(dve_sem, 1)

    return nc


def get_program():
    if "nc" not in _PROGRAM_CACHE:
        _PROGRAM_CACHE["nc"] = build_program()
    return _PROGRAM_CACHE["nc"]


def make_in_maps(x, p_mu, p_logvar):
    """Shard full inputs into per-core input maps (data-parallel over rows)."""
    x = np.asarray(x, dtype=np.float32)
    p_mu = np.asarray(p_mu, dtype=np.float32)
    p_logvar = np.asarray(p_logvar, dtype=np.float32)
    xk = x.reshape(NCORES, D, ROWS)  # core k's slab of flat_x, transposed
    in_maps = []
    for k in range(NCORES):
        rows = slice(ROWS * k, ROWS * (k + 1))
        in_maps.append({
            "xT": np.ascontiguousarray(xk[k]),
            "muT": np.ascontiguousarray(p_mu[rows].T),
            "lvT": np.ascontiguousarray(p_logvar[rows].T),
        })
    return in_maps


def _unpack_stats(stats_arr):
    """(128, 7*NT) device layout -> (7, 512) per-channel stat vectors."""
    out = np.empty((NSTAT, D), dtype=np.float64)
    for s in range(NSTAT):
        sub = stats_arr[:, s * NT : (s + 1) * NT]  # (128, NT); sub[p, t] = v[t*128+p]
        out[s] = sub.T.reshape(D).astype(np.float64)
    return out


def combine(stats_per_core):
    """Host epilogue: all-reduce the per-core stat vectors and form the scalar."""
    tot = np.zeros((NSTAT, D), dtype=np.float64)
    for arr in stats_per_core:
        tot += _unpack_stats(arr)
    A, Bv, Sx, Sx2, Px2, Pxbm, Cv = tot
    Csum = Cv.sum()
    P = Px2.sum() - 2.0 * Pxbm.sum() + Csum       # sum_i sum_d (x-mu)^2*ie
    neg = A @ Sx2 - 2.0 * (Bv @ Sx) + N * Csum    # sum_{i,j} D[i,j]
    loss = (-0.5 * P + 0.5 / N * neg) / N
    return np.float32(loss)


def run_on_device(in_maps, trace=False, **kwargs):
    from concourse.bass_utils import run_bass_kernel_spmd

    nc = get_program()
    return run_bass_kernel_spmd(nc, in_maps, list(range(NCORES)), trace=trace,
                                **kwargs)


def kernel(x, p_mu, p_logvar):
    in_maps = make_in_maps(x, p_mu, p_logvar)
    br = run_on_device(in_maps)
    return combine([r["stats"] for r in br.results])
